# revision 1
# baseline (speedup 1.0000x reference)
"""ClusterNorm1d TRN2 kernel.

Math (per cluster k): mu = mean_b x[b,:,k]; cov = centered second moment;
L = chol(cov + eps I); Z = L^-1 (x - mu).  Output Z transposed back.

Strategy per core (32 clusters): K-sharded across 8 cores, no collectives.
  - stats: bf16 matmuls over a host-prepared [b, (d|1)] tensor, accumulating
    U^T U = [[S, s], [s^T, B]] in fp32 PSUM (32 accumulating matmuls).
  - cov -> W = L^-1 via 4 Newton iterations on the Cholesky manifold:
      P = W A W^T;  C^T = CM o (I - P);  W <- W + C^T^T W
    (CM = triu(1) + 0.5 I).  Converges quadratically; exact-fp32 validated.
  - solve: Z = W x - (W mu) 1^T as float32r matmuls (1 cyc/row @ N=512),
    mean applied as per-partition bias during the PSUM->SBUF copy.
Host supplies x pre-transposed per core as [32, 64, 4096] (f32r) and the
bf16 stats operand [32, 4096, 66] (col 64 = ones, col 65 pad).
"""
import sys
sys.path.insert(0, "/opt/trn_rl_repo")

import numpy as np
import ml_dtypes

import concourse.bass as bass
from concourse import bacc
import concourse.mybir as mybir
import concourse.tile as tile
from concourse.bass_utils import run_bass_kernel_spmd

B, D, K, NCORES = 4096, 64, 256, 8
KL = K // NCORES          # clusters per core
EPS = 1e-4
NB = B // 512             # solve chunks per cluster
AF = mybir.ActivationFunctionType

_cache = {}


def _build_nc(repeat=1):
    nc = bacc.Bacc("TRN2", target_bir_lowering=False, debug=False,
                   num_devices=NCORES)
    d_xs = nc.dram_tensor("xs", [KL, D, B], mybir.dt.float32r,
                          kind="ExternalInput")
    d_xb = nc.dram_tensor("xb", [KL, B, 66], mybir.dt.bfloat16,
                          kind="ExternalInput")
    d_cs = nc.dram_tensor("cs", [D, 4 * D], mybir.dt.float32,
                          kind="ExternalInput")
    d_out = nc.dram_tensor("out", [KL, D, B], mybir.dt.float32,
                           kind="ExternalOutput")

    inv_b = 1.0 / B
    a_cov = 1.0 / (B - 1)
    b_cov = 1.0 / (B * (B - 1.0))

    with tile.TileContext(nc) as tc:
        with tc.tile_pool(name="consts", bufs=1) as consts, \
             tc.tile_pool(name="slab", bufs=2) as slabp, \
             tc.tile_pool(name="upool", bufs=2) as upool, \
             tc.tile_pool(name="zpool", bufs=2) as zpool, \
             tc.tile_pool(name="small", bufs=4) as small, \
             tc.tile_pool(name="wpool", bufs=8) as wpool, \
             tc.tile_pool(name="ps_stat", bufs=2, space="PSUM") as ps_stat, \
             tc.tile_pool(name="ps_small", bufs=4, space="PSUM") as ps_small, \
             tc.tile_pool(name="ps_z", bufs=2, space="PSUM") as ps_z:

            tcs = consts.tile([D, 4 * D], mybir.dt.float32)
            nc.sync.dma_start(out=tcs, in_=d_cs.ap())
            ident = tcs[:, 0:D]
            cmask = tcs[:, D:2 * D]        # triu(1,k=1) + 0.5 I
            chalf = tcs[:, 2 * D:3 * D]    # 0.5 I
            epsi = tcs[:, 3 * D:4 * D]     # EPS * I

            for p0 in range(repeat * (KL // 2)):
                p = p0 % (KL // 2)
                k0, k1 = 2 * p, 2 * p + 1
                # ---- x slab for the pair: [128, 4096] f32r, full-width ----
                slab = slabp.tile([2 * D, B], mybir.dt.float32r)
                nc.sync.dma_start(
                    out=slab,
                    in_=d_xs.ap()[k0:k0 + 2].rearrange("c d b -> (c d) b"))

                zpair = zpool.tile([2 * D, B], mybir.dt.float32)
                outdma_deps = []

                for half, kk in enumerate((k0, k1)):
                    # ---- stats ----
                    ub = upool.tile([128, (B // 128) * 66], mybir.dt.bfloat16)
                    nc.scalar.dma_start(
                        out=ub,
                        in_=d_xb.ap()[kk].rearrange("(p j) c -> p (j c)",
                                                    p=128))
                    ps = ps_stat.tile([D + 1, D + 1], mybir.dt.float32)
                    for j in range(B // 128):
                        sl = ub[:, 66 * j:66 * j + 65]
                        nc.tensor.matmul(ps, sl, sl, start=(j == 0),
                                         stop=(j == B // 128 - 1))
                    st = small.tile([D + 1, D + 1], mybir.dt.float32,
                                    tag="st")
                    nc.scalar.copy(st, ps)

                    # ---- s s^T via K=2 matmul at base 0 ----
                    z2 = small.tile([2, D + 1], mybir.dt.float32, tag="z2")
                    nc.vector.memset(z2, 0.0)
                    nc.scalar.copy(z2[0:1, :], st[D:D + 1, :])
                    pso = ps_small.tile([D, D], mybir.dt.float32, tag="ps64")
                    nc.tensor.matmul(pso, z2[:, 0:D], z2[:, 0:D],
                                     start=True, stop=True)

                    # ---- cov A = S/(B-1) - s s^T/(B(B-1)) + eps I ----
                    t1 = small.tile([D, D], mybir.dt.float32, tag="t1")
                    nc.vector.tensor_scalar_mul(t1, st[0:D, 0:D], a_cov)
                    t2 = small.tile([D, D], mybir.dt.float32, tag="t2")
                    nc.vector.tensor_scalar_mul(t2, pso, b_cov)
                    t3 = small.tile([D, D], mybir.dt.float32, tag="t3")
                    nc.vector.tensor_sub(t3, t1, t2)
                    amat = small.tile([D, D], mybir.dt.float32, tag="amat")
                    nc.vector.tensor_add(amat, t3, epsi)

                    # ---- Newton with W0 = I (A ~ I): it0 analytic ----
                    u1 = small.tile([D, D], mybir.dt.float32, tag="u1")
                    nc.vector.tensor_mul(u1, cmask, amat)
                    ct = small.tile([D, D], mybir.dt.float32, tag="ct")
                    nc.vector.tensor_sub(ct, chalf, u1)
                    psd = ps_small.tile([D, D], mybir.dt.float32, tag="ps64")
                    nc.tensor.matmul(psd, ct, ident, start=True, stop=True)
                    w = wpool.tile([D, D], mybir.dt.float32, tag="w")
                    nc.vector.tensor_add(w, ident, psd)
                    NIT = 4
                    for it in range(1, NIT):
                        pst = ps_small.tile([D, D], mybir.dt.float32,
                                            tag="ps64")
                        nc.tensor.transpose(pst, w, ident)
                        wt = wpool.tile([D, D], mybir.dt.float32, tag="wt")
                        nc.scalar.copy(wt, pst)
                        psh = ps_small.tile([D, D], mybir.dt.float32,
                                            tag="ps64")
                        nc.tensor.matmul(psh, amat, wt, start=True, stop=True)
                        h = small.tile([D, D], mybir.dt.float32, tag="h")
                        nc.scalar.copy(h, psh)
                        psp = ps_small.tile([D, D], mybir.dt.float32,
                                            tag="ps64")
                        nc.tensor.matmul(psp, wt, h, start=True, stop=True)
                        u1 = small.tile([D, D], mybir.dt.float32, tag="u1")
                        nc.vector.tensor_mul(u1, cmask, psp)
                        ct = small.tile([D, D], mybir.dt.float32, tag="ct")
                        nc.vector.tensor_sub(ct, chalf, u1)
                        psd = ps_small.tile([D, D], mybir.dt.float32,
                                            tag="ps64")
                        nc.tensor.matmul(psd, ct, w, start=True, stop=True)
                        wn = wpool.tile([D, D], mybir.dt.float32, tag="w")
                        nc.vector.tensor_add(wn, w, psd)
                        w = wn

                    # ---- final W^T as stacked f32r solve weights [128,64]:
                    #      own half = W^T, other half = 0 (K=128 matmul) ----
                    pst = ps_small.tile([D, D], mybir.dt.float32, tag="ps64")
                    nc.tensor.transpose(pst, w, ident)
                    wtr = wpool.tile([2 * D, D], mybir.dt.float32r, tag="wtr")
                    nc.scalar.copy(wtr[half * D:(half + 1) * D, :], pst)
                    nc.scalar.activation(
                        out=wtr[(1 - half) * D:(2 - half) * D, :],
                        in_=slab[0:D, 0:D], func=AF.Identity, scale=0.0)

                    # ---- v = W mu; bias = -v ----
                    mur = small.tile([2 * D, 2], mybir.dt.float32r, tag="mur")
                    nc.scalar.activation(out=mur, in_=slab[:, 0:2],
                                         func=AF.Identity, scale=0.0)
                    nc.scalar.activation(out=mur[half * D:(half + 1) * D, 0:1],
                                         in_=st[0:D, D:D + 1],
                                         func=AF.Identity, scale=inv_b)
                    psv = ps_small.tile([D, 2], mybir.dt.float32, tag="ps64")
                    nc.tensor.matmul(psv, wtr, mur, start=True, stop=True)
                    biask = small.tile([D, 1], mybir.dt.float32, tag="biask")
                    nc.scalar.activation(out=biask, in_=psv[:, 0:1],
                                         func=AF.Identity, scale=-1.0)

                    # ---- solve: Z = W x + bias ----
                    for j in range(NB):
                        psz = ps_z.tile([D, 512], mybir.dt.float32, tag="psz")
                        nc.tensor.matmul(
                            psz, wtr,
                            slab[:, 512 * j: 512 * (j + 1)],
                            start=True, stop=True)
                        dst = zpair[half * D:(half + 1) * D,
                                    512 * j:512 * (j + 1)]
                        if half == 0:
                            cp = nc.scalar.activation(out=dst, in_=psz,
                                                      func=AF.Identity,
                                                      bias=biask)
                        else:
                            cp = nc.vector.tensor_scalar_add(dst, psz, biask)
                        outdma_deps.append(cp)

                nc.sync.dma_start(
                    out=d_out.ap()[k0:k0 + 2].rearrange("c d b -> (c d) b"),
                    in_=zpair)

    nc.finalize()
    return nc


def _make_consts():
    ident = np.eye(D, dtype=np.float32)
    cmask = np.triu(np.ones((D, D), np.float32), 1) + 0.5 * ident
    chalf = 0.5 * ident
    epsi = EPS * ident
    return np.concatenate([ident, cmask, chalf, epsi], axis=1)


def _prep_inputs(x):
    """x: [B, D, K] fp32 -> per-core input dicts."""
    consts = _make_consts()
    in_maps = []
    for c in range(NCORES):
        ks = slice(c * KL, (c + 1) * KL)
        xs = np.ascontiguousarray(x[:, :, ks].transpose(2, 1, 0))  # [KL, D, B]
        xt = xs.transpose(0, 2, 1)                                  # [KL, B, D]
        xb = np.empty((KL, B, 66), dtype=ml_dtypes.bfloat16)
        xb[:, :, 0:D] = xt.astype(ml_dtypes.bfloat16)
        xb[:, :, D] = np.float32(1.0)
        xb[:, :, D + 1] = np.float32(0.0)
        in_maps.append({"xs": xs, "xb": xb, "cs": consts})
    return in_maps


def _run(x, trace=False):
    if "nc" not in _cache:
        _cache["nc"] = _build_nc()
    nc = _cache["nc"]
    in_maps = _prep_inputs(np.asarray(x, dtype=np.float32))
    res = run_bass_kernel_spmd(nc, in_maps, core_ids=list(range(NCORES)),
                               trace=trace)
    out = np.empty((B, D, K), dtype=np.float32)
    for c in range(NCORES):
        ks = slice(c * KL, (c + 1) * KL)
        out[:, :, ks] = res.results[c]["out"].transpose(2, 1, 0)
    return out, res


def kernel(x):
    out, _ = _run(x, trace=False)
    return out



# revision 6
# speedup vs baseline: 3.6377x; 3.6377x over previous
"""ClusterNorm1d TRN2 kernel (v1: phase-restructured, bf16, NIT=2).

Math (per cluster k): mu = mean_b x[b,:,k]; cov = centered second moment;
L = chol(cov + eps I); Z = L^-1 (x - mu).

Strategy per core (32 clusters = 16 pairs; K-sharded across 8 cores):
  Phase 1 (stats): dense PE stream -- per cluster 33 accumulating bf16
    matmuls build U^T U (+ eps*(B-1) I folded in via a rank-64 matmul of
    a const diagonal), U = [x_chunk | 1].  Results parked in st_all.
  Phase 2 (Newton, batched by step over 8-pair groups): clusters stacked
    in pairs on 128 partitions; per step the 16 quadrant matmuls
    (tile_position (0,0)/(64,64) via base-partition slices) run nearly
    concurrently and DVE ops process 2 clusters per op.  Both W and W^T
    are maintained (2 matmuls) instead of PE transposes.  NIT=2 total
    Newton steps (it0 analytic + 1), validated rel_err ~3e-3.
  Phase 3 (solve): Z = W x - (W mu) 1^T as bf16 quadrant matmuls at
    N=512, mean applied as per-partition bias during PSUM->SBUF copies
    (alternating ACT/DVE).  f32 output slab DMA'd per pair.
HBM traffic: xs bf16 16.8MB + xb bf16 17.3MB in, 33.5MB f32 out.
"""
import sys
sys.path.insert(0, "/opt/trn_rl_repo")

import numpy as np
import ml_dtypes

import concourse.bass as bass
from concourse import bacc
import concourse.mybir as mybir
import concourse.tile as tile
from concourse.bass_utils import run_bass_kernel_spmd

B, D, K, NCORES = 4096, 64, 256, 8
KL = K // NCORES          # clusters per core
NP = KL // 2              # pairs per core
GRP = 8                   # pairs per newton/solve group
EPS = 1e-4
NB = B // 512             # solve chunks per pair
NJ = B // 128             # stats chunks per cluster
AF = mybir.ActivationFunctionType
OP = mybir.AluOpType

_cache = {}


def _build_nc():
    nc = bacc.Bacc("TRN2", target_bir_lowering=False, debug=False,
                   num_devices=NCORES)
    d_xs = nc.dram_tensor("xs", [KL, D, B], mybir.dt.bfloat16,
                          kind="ExternalInput")
    d_xb = nc.dram_tensor("xb", [NP, 128, 2 * NJ * 66], mybir.dt.bfloat16,
                          kind="ExternalInput")
    d_cs = nc.dram_tensor("cs", [2 * D, 3 * D], mybir.dt.float32,
                          kind="ExternalInput")
    d_eb = nc.dram_tensor("eb", [D, 66], mybir.dt.bfloat16,
                          kind="ExternalInput")
    d_out = nc.dram_tensor("out", [KL, D, B], mybir.dt.float32,
                           kind="ExternalOutput")

    inv_b = 1.0 / B
    a_cov = 1.0 / (B - 1)
    b_cov = 1.0 / (B * (B - 1.0))
    STW = D + 1           # st tile width per cluster

    with tile.TileContext(nc) as tc:
        with tc.tile_pool(name="consts", bufs=1) as consts, \
             tc.tile_pool(name="slabp", bufs=9) as slabp, \
             tc.tile_pool(name="upool", bufs=3) as upool, \
             tc.tile_pool(name="zpool", bufs=3) as zpool, \
             tc.tile_pool(name="pers", bufs=GRP) as pers, \
             tc.tile_pool(name="step", bufs=GRP) as step, \
             tc.tile_pool(name="small", bufs=4) as small, \
             tc.tile_pool(name="ps_stat", bufs=2, space="PSUM") as ps_stat, \
             tc.tile_pool(name="ps_small", bufs=3, space="PSUM") as ps_small, \
             tc.tile_pool(name="ps_z", bufs=3, space="PSUM") as ps_z:

            tcs = consts.tile([2 * D, 3 * D], mybir.dt.float32)
            nc.sync.dma_start(out=tcs, in_=d_cs.ap())
            id2 = tcs[:, 0:D]
            cm2 = tcs[:, D:2 * D]          # triu(1,k=1) + 0.5 I, stacked
            ch2 = tcs[:, 2 * D:3 * D]      # 0.5 I, stacked
            teb = consts.tile([D, 66], mybir.dt.bfloat16)
            nc.sync.dma_start(out=teb, in_=d_eb.ap())
            ebs = teb[:, 0:D + 1]          # sqrt(eps*(B-1)) I | 0

            # st_all[:, 65k : 65k+65] = U_k^T U_k + eps*(B-1) I' ; +pad col
            st_all = consts.tile([STW, STW * KL + 1], mybir.dt.float32)

            # ---- prefetch all solve slabs (sync/SP ring only) ----
            slabs = []
            for p in range(NP):
                sl = slabp.tile([2 * D, B], mybir.dt.bfloat16)
                nc.sync.dma_start(
                    out=sl,
                    in_=d_xs.ap()[2 * p:2 * p + 2].rearrange(
                        "c d b -> (c d) b"))
                slabs.append(sl)

            # ---- phase 1: stats ----
            for p in range(NP):
                ub = upool.tile([128, 2 * NJ * 66], mybir.dt.bfloat16)
                nc.scalar.dma_start(out=ub, in_=d_xb.ap()[p])
                for half in range(2):
                    kk = 2 * p + half
                    off = half * NJ * 66
                    ps = ps_stat.tile([STW, STW], mybir.dt.float32)
                    for j in range(NJ):
                        sl = ub[:, off + 66 * j: off + 66 * j + STW]
                        nc.tensor.matmul(ps, sl, sl, start=(j == 0),
                                         stop=False)
                    nc.tensor.matmul(ps, ebs, ebs, start=False, stop=True)
                    nc.scalar.copy(st_all[:, STW * kk:STW * (kk + 1)], ps)

            # ---- phases 2+3 per group ----
            for g in range(NP // GRP):
                pairs = range(g * GRP, (g + 1) * GRP)

                # - step A: amat (cov + eps, pair-stacked) -
                amats = {}
                z2s = {}
                for p in pairs:
                    k0, k1 = 2 * p, 2 * p + 1
                    ams = step.tile([2 * D, D], mybir.dt.float32, tag="ams")
                    nc.scalar.activation(
                        out=ams[0:D, :], in_=st_all[0:D, STW * k0:STW * k0 + D],
                        func=AF.Identity, scale=a_cov)
                    nc.scalar.activation(
                        out=ams[D:2 * D, :],
                        in_=st_all[0:D, STW * k1:STW * k1 + D],
                        func=AF.Identity, scale=a_cov)
                    z2 = step.tile([2 * D, D], mybir.dt.float32, tag="z2")
                    nc.scalar.copy(z2[0:1, :],
                                   st_all[D:D + 1, STW * k0:STW * k0 + D])
                    nc.scalar.copy(z2[D:D + 1, :],
                                   st_all[D:D + 1, STW * k1:STW * k1 + D])
                    z2s[p] = z2
                    pso = ps_small.tile([2 * D, D], mybir.dt.float32,
                                        tag="ps64")
                    nc.tensor.matmul(pso[0:D, :], z2[0:1, :], z2[0:1, :],
                                     start=True, stop=True)
                    nc.tensor.matmul(pso[D:2 * D, :], z2[D:D + 1, :],
                                     z2[D:D + 1, :], start=True, stop=True)
                    am = step.tile([2 * D, D], mybir.dt.float32, tag="amat")
                    nc.vector.scalar_tensor_tensor(
                        out=am, in0=pso, scalar=-b_cov, in1=ams,
                        op0=OP.mult, op1=OP.add)
                    amats[p] = am

                # - step B: it0 (W0 = I): ct = 0.5I - cm o A -
                w1s = {}
                wt1s = {}
                for p in pairs:
                    am = amats[p]
                    u1 = step.tile([2 * D, D], mybir.dt.float32, tag="u1")
                    nc.vector.tensor_mul(u1, cm2, am)
                    ct = step.tile([2 * D, D], mybir.dt.float32, tag="ct")
                    nc.vector.tensor_sub(ct, ch2, u1)
                    psd = ps_small.tile([2 * D, D], mybir.dt.float32,
                                        tag="ps64")
                    nc.tensor.matmul(psd[0:D, :], ct[0:D, :], id2[0:D, :],
                                     start=True, stop=True)
                    nc.tensor.matmul(psd[D:2 * D, :], ct[D:2 * D, :],
                                     id2[D:2 * D, :], start=True, stop=True)
                    w1 = step.tile([2 * D, D], mybir.dt.float32, tag="w1")
                    nc.vector.tensor_add(w1, id2, psd)
                    wt1 = step.tile([2 * D, D], mybir.dt.float32, tag="wt1")
                    nc.vector.tensor_add(wt1, id2, ct)
                    w1s[p] = w1
                    wt1s[p] = wt1

                # - step C: one coupled Newton iteration; final W^T only -
                hs = {}
                for p in pairs:
                    am, wt1 = amats[p], wt1s[p]
                    psh = ps_small.tile([2 * D, D], mybir.dt.float32,
                                        tag="ps64")
                    nc.tensor.matmul(psh[0:D, :], am[0:D, :], wt1[0:D, :],
                                     start=True, stop=True)
                    nc.tensor.matmul(psh[D:2 * D, :], am[D:2 * D, :],
                                     wt1[D:2 * D, :], start=True, stop=True)
                    h2 = step.tile([2 * D, D], mybir.dt.float32, tag="h2")
                    nc.scalar.copy(h2, psh)
                    hs[p] = h2
                wtbs = {}
                for p in pairs:
                    wt1, h2, w1 = wt1s[p], hs[p], w1s[p]
                    psp = ps_small.tile([2 * D, D], mybir.dt.float32,
                                        tag="ps64")
                    nc.tensor.matmul(psp[0:D, :], wt1[0:D, :], h2[0:D, :],
                                     start=True, stop=True)
                    nc.tensor.matmul(psp[D:2 * D, :], wt1[D:2 * D, :],
                                     h2[D:2 * D, :], start=True, stop=True)
                    u1 = step.tile([2 * D, D], mybir.dt.float32, tag="u1")
                    nc.vector.tensor_mul(u1, cm2, psp)
                    ct = step.tile([2 * D, D], mybir.dt.float32, tag="ct")
                    nc.vector.tensor_sub(ct, ch2, u1)
                    pst = ps_small.tile([2 * D, D], mybir.dt.float32,
                                        tag="ps64")
                    nc.tensor.matmul(pst[0:D, :], w1[0:D, :], ct[0:D, :],
                                     start=True, stop=True)
                    nc.tensor.matmul(pst[D:2 * D, :], w1[D:2 * D, :],
                                     ct[D:2 * D, :], start=True, stop=True)
                    wt = step.tile([2 * D, D], mybir.dt.float32, tag="wt")
                    nc.vector.tensor_add(wt, wt1, pst)
                    wtb = pers.tile([2 * D, D], mybir.dt.bfloat16, tag="wtb")
                    nc.vector.tensor_copy(wtb, wt)
                    wtbs[p] = wtb

                # - phase 3: solve -
                for p in pairs:
                    k0, k1 = 2 * p, 2 * p + 1
                    wtb, slab = wtbs[p], slabs[p]
                    mub = small.tile([2 * D, 2], mybir.dt.bfloat16,
                                     tag="mub")
                    nc.scalar.activation(
                        out=mub[0:D, :],
                        in_=st_all[0:D, STW * k0 + D:STW * k0 + D + 2],
                        func=AF.Identity, scale=inv_b)
                    nc.scalar.activation(
                        out=mub[D:2 * D, :],
                        in_=st_all[0:D, STW * k1 + D:STW * k1 + D + 2],
                        func=AF.Identity, scale=inv_b)
                    psv = ps_small.tile([2 * D, 2], mybir.dt.float32,
                                        tag="ps64")
                    nc.tensor.matmul(psv[0:D, :], wtb[0:D, :], mub[0:D, :],
                                     start=True, stop=True)
                    nc.tensor.matmul(psv[D:2 * D, :], wtb[D:2 * D, :],
                                     mub[D:2 * D, :], start=True, stop=True)
                    biask = small.tile([2 * D, 1], mybir.dt.float32,
                                       tag="biask")
                    nc.scalar.activation(out=biask, in_=psv[:, 0:1],
                                         func=AF.Identity, scale=-1.0)

                    zs = zpool.tile([2 * D, B], mybir.dt.float32)
                    for j in range(NB):
                        psz = ps_z.tile([2 * D, 512], mybir.dt.float32)
                        nc.tensor.matmul(psz[0:D, :], wtb[0:D, :],
                                         slab[0:D, 512 * j:512 * (j + 1)],
                                         start=True, stop=True)
                        nc.tensor.matmul(psz[D:2 * D, :], wtb[D:2 * D, :],
                                         slab[D:2 * D, 512 * j:512 * (j + 1)],
                                         start=True, stop=True)
                        dst = zs[:, 512 * j:512 * (j + 1)]
                        if j % 2 == 0:
                            nc.scalar.activation(out=dst, in_=psz,
                                                 func=AF.Identity,
                                                 bias=biask)
                        else:
                            nc.vector.tensor_scalar_add(dst, psz, biask)
                    nc.gpsimd.dma_start(
                        out=d_out.ap()[k0:k0 + 2].rearrange(
                            "c d b -> (c d) b"),
                        in_=zs)

    nc.finalize()
    return nc


def _make_consts():
    ident = np.eye(D, dtype=np.float32)
    cmask = np.triu(np.ones((D, D), np.float32), 1) + 0.5 * ident
    chalf = 0.5 * ident
    blk = np.concatenate([ident, cmask, chalf], axis=1)      # [D, 3D]
    return np.concatenate([blk, blk], axis=0)                # [2D, 3D]


def _make_eb():
    eb = np.zeros((D, 66), dtype=ml_dtypes.bfloat16)
    eb[:, 0:D] = (np.sqrt(EPS * (B - 1)) *
                  np.eye(D, dtype=np.float32)).astype(ml_dtypes.bfloat16)
    return eb


def _prep_inputs(x):
    """x: [B, D, K] fp32 -> per-core input dicts."""
    consts = _make_consts()
    eb = _make_eb()
    in_maps = []
    for c in range(NCORES):
        ks = slice(c * KL, (c + 1) * KL)
        xc = x[:, :, ks]
        xs = np.ascontiguousarray(xc.transpose(2, 1, 0)).astype(
            ml_dtypes.bfloat16)                                 # [KL, D, B]
        xt = xc.transpose(2, 0, 1)                              # [KL, B, D]
        xb = np.zeros((KL, B, 66), dtype=ml_dtypes.bfloat16)
        xb[:, :, 0:D] = xt.astype(ml_dtypes.bfloat16)
        xb[:, :, D] = np.float32(1.0)
        # repack to per-pair SBUF layout: [NP, 128, 2*NJ*66]
        xb = np.ascontiguousarray(
            xb.reshape(NP, 2, NJ, 128, 66).transpose(0, 3, 1, 2, 4)
            .reshape(NP, 128, 2 * NJ * 66))
        in_maps.append({"xs": xs, "xb": xb, "cs": consts, "eb": eb})
    return in_maps


def _run(x, trace=False):
    if "nc" not in _cache:
        _cache["nc"] = _build_nc()
    nc = _cache["nc"]
    in_maps = _prep_inputs(np.asarray(x, dtype=np.float32))
    res = run_bass_kernel_spmd(nc, in_maps, core_ids=list(range(NCORES)),
                               trace=trace)
    out = np.empty((B, D, K), dtype=np.float32)
    for c in range(NCORES):
        ks = slice(c * KL, (c + 1) * KL)
        out[:, :, ks] = res.results[c]["out"].transpose(2, 1, 0)
    return out, res


def kernel(x):
    out, _ = _run(x, trace=False)
    return out
